# revision 38
# baseline (speedup 1.0000x reference)
"""MoE expert-network kernel for 8 Trainium2 NeuronCores.

Strategy: expert parallelism (E == n_cores == 8). The host dispatches each
token to its expert's core (an all-to-all in numpy), folds the inference-mode
BatchNorm into the expert weights/bias, and each core runs one dense
[cap, 512] @ [512, 512] GEMM fused with bias + SiLU via the activation engine.

All device tensors are laid out host-side as the exact SBUF tile images
(128-partition-major, block-contiguous per token tile) so every DMA is a
plain 2D contiguous copy with multi-KB lines.

Per-core device program (identical on all cores, SPMD):
  inputs : xs [128, KC*cap]      fp16 - token tiles, partition-major blocks
           wx [128, MC*KC*128 + KC*s0] fp16 - BN-folded weights (m-major
                                        blocks) ++ the first x tile
           bs [128, MC]          fp32 - BN-folded bias tile image
  output : os [128, MC*cap]      fp16 - silu(x @ W + b), (tile, m)-major
x is shipped fp16 (~2e-4 rel error, halves the dominant stream); the host
scatters the result back into the full [B, 512] fp32 output.

Pipeline design notes (from perfetto traces):
  - exec time is measured first-user-instruction -> last-instruction; the
    epilogue carries a fixed ~10.3us serial cost after the last SILU
    (terminal-store completion ~2.7us + TileContext teardown ~1.3us + the
    walrus epilogue's semaphore sweep, 51 sem-clears at a constant 115ns
    on the Tensor sequencer + final barrier). The sweep pace is NOT
    HAM-dependent (sequencers run at a fixed clock), so the only lever is
    finishing the compute+drain early;
  - the weights + first x tile ride ONE DMA at the head of the sync ring:
    every DMA has ~2us trigger-to-first-completion latency and concurrent
    descriptors round-robin the 16 engines (starving earlier transfers'
    tails), so a single fused leading image beats any split/racing scheme
    once the PE is at 2.4GHz (a 128KB weight block lasts only 213ns, and
    any split also pushes the x stream later on the ring);
  - leading dummy matmuls on a zeroed tile bridge the entire wx DMA wait
    (~5.7us) and pre-pay the full HAM cold window (~3.4us of sustained
    activity -> 2.4GHz), so every real matmul runs at full clock;
  - 256/512 lead tiles then 1024s: small enough that the first matmul
    group waits only ~5.7us for the 768KB wx image, large enough that the
    x stream and the ACT engine's ~450ns/SILU overhead keep pace with the
    warm PE; a 512-token tail tile keeps the final ACT->store chain short
    and paced with the GpSimd Q7's ~700ns store-trigger emission;
  - a dummy SILU pulls the ~2.6us ACT table loads off the critical path;
  - psum tiles span 2 banks: one SILU instruction reads up to 1024
    columns, halving the ACT engine's 352-cycle per-op overhead;
  - stores are per (m-pair, tile) slices fired right after each SILU:
    smooth out-stream without flooding the GpSimd SWDGE queue (~1us
    descriptor emission per store op); the last tile's stores ride per-m,
    with the terminal one on the scalar-free sync HWDGE ring for low
    completion latency.
"""

import sys

for _p in ("/opt/trn_rl_repo",):
    if _p not in sys.path:
        sys.path.append(_p)

import numpy as np

import concourse.bass as bass
import concourse.mybir as mybir
import concourse.tile as tile
from concourse import bacc
from concourse.bass_utils import run_bass_kernel_spmd

B = 32768
IN = 512
HID = 512
E = 8
NCORES = 8
EPS = 1e-5
P = 128  # SBUF partitions
NT = 512  # matmul moving-dim chunk (one fp32 PSUM bank)

KC = IN // P  # contraction chunks
MC = HID // P  # output-feature chunks
NWARM = 7  # leading HAM-prewarm dummy matmuls (N=512, cold ~530ns each):
# they span ~1.3us..~5.0us, bridging the first weight-chunk DMA wait with
# NO idle gap: the HAM's ~3.4us busy window needs contiguous activity (a
# ~0.6us gap before the first real matmul resets it and the whole ramp
# re-runs at 1.2GHz, ~+1.7us). N=512 keeps the PE-array
# duty ~80% (the interleaved LDWEIGHTS don't count as activity; N=128
# dummies sit at ~50% duty and the HAM un-throttle fires late/erratically).


def plan_sizes(cap: int) -> list:
    """Token-tile sizes: 256/512 lead tiles (fast pipeline ramp without
    outrunning the ACT engine), a 512 tail tile (short final ACT->store
    chain), 1024-wide tiles in the middle."""
    if cap < 1280:  # not reachable for the real token distribution
        return [min(512, cap - o) for o in range(0, cap, 512)]
    # 512/512 lead tiles: tile 0 streams in as four k-chunks (see
    # build_bass), and >=512-token tiles keep the ACT engine's ~450ns/SILU
    # fixed overhead off the warm PE's back.
    sizes = [512, 512]
    # Reserve a 512-token tail tile: its per-m SILUs (~720ns) are SHORTER
    # than their matmul groups (~852ns), so the ACT engine tracks the PE
    # through the final tile, and its 720ns SILU pace matches the GpSimd
    # Q7's ~700ns store-trigger emission (a 256 tail bunches the final
    # stores behind the Q7 and delays the drain by >1us).
    rem = cap - 1024 - 512
    while rem >= 1024:
        sizes.append(1024)
        rem -= 1024
    if rem:
        sizes.append(rem)
    sizes.append(512)
    return sizes


def build_bass(cap: int, act: str = "silu") -> bass.Bass:
    nc = bacc.Bacc(
        "TRN2",
        target_bir_lowering=False,
        debug=False,
        enable_asserts=False,
        num_devices=NCORES,
    )
    f32 = mybir.dt.float32
    f16 = mybir.dt.float16

    tiles = []
    n0 = 0
    for s in plan_sizes(cap):
        tiles.append((n0, s))
        n0 += s
    s0 = tiles[0][1]
    CW = MC * P + s0  # columns per k-chunk: [W_k (all m) | x0_k]

    # wx = KC k-chunks [W[k,:,m=0..3] | x0[k]], four racing DMAs: tile 0
    # runs k-outer and accumulates all MC psums across chunk arrivals, so
    # the first matmul starts after one 256KB chunk (~4.2us) instead of
    # the whole 1MB image (~6.3us). Weight slices for every later tile are
    # read from the same chunk tiles.
    xs = nc.dram_tensor("xs", [P, KC * cap], f16, kind="ExternalInput").ap()
    wx = nc.dram_tensor("wx", [P, KC * CW], f16, kind="ExternalInput").ap()
    bs = nc.dram_tensor("bs", [P, MC], f32, kind="ExternalInput").ap()
    os_ = nc.dram_tensor("os", [P, MC * cap], f16, kind="ExternalOutput").ap()

    with tile.TileContext(nc) as tc:
        with (
            tc.tile_pool(name="wpool", bufs=1) as wpool,
            tc.tile_pool(name="xpool", bufs=4) as xpool,
            tc.tile_pool(name="opool", bufs=3) as opool,
            # Split PSUM pools (8 banks total): 4-deep rotation for <=512
            # token tiles (short m-groups need slack against the ACT
            # engine's per-SILU overhead) and 2-deep for the big tiles
            # (their 1.7us m-groups give the ACT plenty of headroom).
            tc.tile_pool(name="pps", bufs=4, space="PSUM") as pps,
            tc.tile_pool(name="ppb", bufs=2, space="PSUM") as ppb,
        ):
            # The four k-chunks ride the sync ring at the head of the
            # FIFO; the PE's k-outer loop on tile 0 paces their ~0.8us
            # arrival spacing.
            wk = []
            for k in range(KC):
                wt = wpool.tile([P, CW], f16, tag=f"wk{k}", name=f"wk{k}")
                nc.sync.dma_start(out=wt, in_=wx[:, k * CW : (k + 1) * CW])
                wk.append(wt)
            bt = wpool.tile([P, MC], f32, tag="bt", name="bt")
            nc.scalar.dma_start(out=bt, in_=bs)

            # Leading dummy matmuls on a zeroed scratch tile bridge the wx
            # DMA wait and pre-pay the HAM clock-gate window.
            warm = wpool.tile([P, NT], f16, tag="warm", name="warm")
            nc.gpsimd.memset(warm, 0.0)
            if act == "silu":
                # Tiny dummy SILU: walrus places the ACT table load right
                # before the first activation on each path; doing one now
                # (on the idle ACT engine, during the DMA ramp) keeps the
                # ~2.6us table loads off the steady-state critical path.
                sact = wpool.tile([P, 16], f16, tag="sact", name="sact")
                nc.scalar.activation(
                    sact, warm[:, :16], mybir.ActivationFunctionType.Silu
                )
            wps = pps.tile([P, NT], f32, tag="ps", name="wps")
            for _ in range(NWARM):
                nc.tensor.matmul(
                    wps, lhsT=warm[:, :P], rhs=warm, start=True, stop=True
                )

            for ti, (n0, nt) in enumerate(tiles):
                if ti == 0:
                    xt = None  # tile 1's tokens live inside the k-chunks
                else:
                    xt = xpool.tile([P, KC, nt], f16, tag="xt", name="xt")
                    nc.sync.dma_start(
                        out=xt, in_=xs[:, KC * n0 : KC * (n0 + nt)]
                    )
                ot = opool.tile([P, MC, nt], f16, tag="ot", name="ot")
                ng = -(-nt // NT)  # 512-chunks in this tile (<= 2)
                if ti == 0:
                    # k-outer: all MC accumulations open at once in the big
                    # psum pool (two m's per 2-bank tile), each chunk's 4
                    # matmuls issue as soon as that chunk's DMA lands. The
                    # small pool stays free for tile 1, so tile 0's
                    # end-bunched SILUs never backpressure the PE.
                    pss = [
                        ppb.tile([P, 2 * NT], f32, tag="ps", name="ps0")
                        for _ in range(-(-MC // 2))
                    ]
                    for k in range(KC):
                        for m in range(MC):
                            col = (m % 2) * NT
                            nc.tensor.matmul(
                                pss[m // 2][:, col : col + nt],
                                lhsT=wk[k][:, m * P : (m + 1) * P],
                                rhs=wk[k][:, MC * P : MC * P + nt],
                                start=(k == 0),
                                stop=(k == KC - 1),
                            )
                for m in range(MC):
                    if ti == 0:
                        ps = pss[m // 2]
                        pview = ps[:, (m % 2) * NT : (m % 2) * NT + nt]
                    else:
                        # ng PSUM banks; one SILU reads the whole span
                        pool = pps if ng == 1 else ppb
                        ps = pool.tile([P, ng * NT], f32, tag="ps", name="ps")
                        for g in range(ng):
                            off = g * NT
                            ns = min(NT, nt - off)
                            for k in range(KC):
                                nc.tensor.matmul(
                                    ps[:, off : off + ns],
                                    lhsT=wk[k][:, m * P : (m + 1) * P],
                                    rhs=xt[:, k, off : off + ns],
                                    start=(k == 0),
                                    stop=(k == KC - 1),
                                )
                        pview = ps[:, :nt]
                    osl = ot[:, m]
                    if act == "silu":
                        nc.scalar.activation(
                            osl,
                            pview,
                            mybir.ActivationFunctionType.Silu,
                            bias=bt[:, m : m + 1],
                        )
                    else:
                        # CoreSim has no Silu: Identity+Sigmoid+mul
                        yt = opool.tile([P, nt], f32, tag="yt", name="yt")
                        nc.scalar.activation(
                            yt,
                            pview,
                            mybir.ActivationFunctionType.Identity,
                            bias=bt[:, m : m + 1],
                        )
                        st = opool.tile([P, nt], f32, tag="st", name="st")
                        nc.scalar.activation(
                            st,
                            pview,
                            mybir.ActivationFunctionType.Sigmoid,
                            bias=bt[:, m : m + 1],
                        )
                        nc.vector.tensor_mul(osl, yt, st)
                    # Store m-pairs (after the m=1 / m=3 SILUs): smooth
                    # out-stream on the (otherwise idle) GpSimd SWDGE ring
                    # without flooding the Q7 descriptor queue (~1us
                    # emission per store op). The last tile stores per-m on
                    # scalar HWDGE: lower completion latency, and the
                    # terminal transfer (the one the exit drain waits on)
                    # is a quarter the size.
                    if ti == len(tiles) - 1:
                        # Per-m stores spread the SWDGE retire over the
                        # tile's SILU window (a fused late store retires
                        # ~0.5us later); the terminal m3 slice rides the
                        # idle sync HWDGE ring for low completion latency.
                        out_eng = nc.sync if m == MC - 1 else nc.gpsimd
                        out_eng.dma_start(
                            out=os_[:, MC * n0 + m * nt : MC * n0 + (m + 1) * nt],
                            in_=osl,
                        )
                    elif m % 2 == 1:
                        nc.gpsimd.dma_start(
                            out=os_[
                                :, MC * n0 + (m - 1) * nt : MC * n0 + (m + 1) * nt
                            ],
                            in_=ot[:, m - 1 : m + 1],
                        )

    nc.compile()
    return nc


def prepare(inputs: dict) -> tuple:
    x = np.ascontiguousarray(np.asarray(inputs["x"], dtype=np.float32))
    idx = np.asarray(inputs["expert_indices"]).astype(np.int64)
    ew = np.asarray(inputs["expert_weights"], dtype=np.float32)
    eb = np.asarray(inputs["expert_biases"], dtype=np.float32)
    gw = np.asarray(inputs["bn_weights"], dtype=np.float32)
    gb = np.asarray(inputs["bn_biases"], dtype=np.float32)
    rm = np.asarray(inputs["running_mean"], dtype=np.float32)
    rv = np.asarray(inputs["running_var"], dtype=np.float32)

    # Fold inference BN into the expert weight/bias:
    #   y = (x @ W + eb - rm) * gw/sqrt(rv+eps) + gb = x @ (W*s) + (eb-rm)*s + gb
    s = gw / np.sqrt(rv + EPS)
    wf = ew * s[:, None, :]
    bf = (eb - rm) * s + gb

    perms = [np.nonzero(idx == e)[0] for e in range(E)]
    counts = [len(p) for p in perms]
    cap = max(512, -(-max(counts) // P) * P)
    tiles = []
    n0 = 0
    for t in plan_sizes(cap):
        tiles.append((n0, t))
        n0 += t

    in_maps = []
    for e in range(E):
        xT = np.zeros((IN, cap), dtype=np.float16)
        if counts[e]:
            xT[:, : counts[e]] = x[perms[e]].T.astype(np.float16)
        xv = xT.reshape(KC, P, cap)
        xs = np.empty((P, KC * cap), dtype=np.float16)
        for n0, nt in tiles:
            xs[:, KC * n0 : KC * (n0 + nt)] = (
                xv[:, :, n0 : n0 + nt].transpose(1, 0, 2).reshape(P, KC * nt)
            )
        # k-chunked leading image: per contraction chunk k, the m-major
        # weight slice ws_k[p, m*P + j] = W[k*P + p, m*P + j] fused with
        # the first x tile's k-th slice
        s0 = tiles[0][1]
        wk4 = wf[e].astype(np.float16).reshape(KC, P, MC * P)
        wx = np.concatenate(
            [
                np.concatenate([wk4[k], xs[:, k * s0 : (k + 1) * s0]], axis=1)
                for k in range(KC)
            ],
            axis=1,
        )
        bs = np.ascontiguousarray(bf[e].reshape(MC, P).T)
        in_maps.append({"xs": xs, "wx": np.ascontiguousarray(wx), "bs": bs})
    return cap, tiles, perms, counts, in_maps


def combine(results: list, cap, tiles, perms, counts) -> np.ndarray:
    out = np.empty((B, HID), dtype=np.float32)
    for e in range(E):
        if not counts[e]:
            continue
        ob = results[e]["os"]
        oT = np.empty((HID, cap), dtype=np.float32)
        for n0, nt in tiles:
            # per-(tile, m) blocks: [P, nt] at column MC*n0 + m*nt
            oT[:, n0 : n0 + nt] = (
                ob[:, MC * n0 : MC * (n0 + nt)]
                .reshape(P, MC, nt)
                .transpose(1, 0, 2)
                .reshape(HID, nt)
            )
        out[perms[e]] = oT[:, : counts[e]].T
    return out


def kernel(**inputs) -> np.ndarray:
    cap, tiles, perms, counts, in_maps = prepare(inputs)
    nc = build_bass(cap)
    res = run_bass_kernel_spmd(nc, in_maps, core_ids=list(range(NCORES)))
    return combine(res.results, cap, tiles, perms, counts)
